# revision 1
# baseline (speedup 1.0000x reference)
"""Trainium2 Bass kernel for the CNN-MAD per-class DTW transport cost.

Math (reference):
  mat_cost[n, j] = C1[n] + C2[c_n, j] - 2*C3[n, j],  c_n = classes[n]
    C1[n]    = sum_{t,d} rowsum[c_n, t] * X[n,t,d]^2
    C2[c, j] = sum_{p,d} colsum[c, p]  * Y[j,p,d]^2
    C3[n, j] = sum_{p,d} XW[n,p,d] * Y[j,p,d],  XW = pi_c.T @ X (warp)

Sharding: 4x2 grid. Core (r, cj) owns the samples of classes {2r, 2r+1}
(zero-padded to cap1 rows per class, NL = 2*cap1) and the j-half
[512*cj, 512*(cj+1)).  One SPMD program for all 8 cores; per-core class
structure enters only through data (pis/aux tensors), all inputs are
host-quantized to fp8e4 (values << 240, encodings match e4m3).

All contractions run on the PE at fp8 DoubleRow rate (0.5 cyc/row):
colsum/rowsum as ones-contractions of pi; warp XW = pi.T @ X; C3 =
xwt.T @ ytl over k=(pc*8+d, p); C2 folds colsum*sum_d Y^2 into one DR
chain against squared Y with the colsum replicated over d; C1 likewise
contracts rowsum over (t,tc,d) k-tiles directly against squared X; the
per-sample C1 and per-class C2 rows are added by two fp16 rank-2
augmentation matmuls per 128-row block into the same psum group.
Squares are fp8 elementwise work split across ACT/DVE/Pool.  Details:
  - piS+piT merged into one input DMA; ytl chunks go via Pool SWDGE so
    the HWDGE pipeline only carries pis/xt2/aux (earlier last-input).
  - rowsum computed right after colsum (pi arrives first).
  - aug folded to ONE rank-4 fp16 matmul per n-block using a host-
    shipped aux tensor [4, NL+512]: rows0-1 ind, rows2-3 zeros (augB
    DMA'd in at runtime) | cols NL:: rows0-1 c2row (evac'd), rows2-3
    ones (shipped).
  - XW/out evacs spread across ACT/DVE/Pool.
  - 3 per-block output DMAs, pipelined.
"""

import sys

sys.path.insert(0, "/opt/trn_rl_repo")

import numpy as np

N, NY, T, TP, D, C = 1024, 1024, 256, 256, 8, 8
NCORES = 8
NYL = 512  # j columns per core

_cache = {}

POOL_PSUM = False  # Pool engine reads PSUM for some evacs


def _build(cap1):
    import concourse.bacc as bacc
    import concourse.mybir as mybir
    import concourse.tile as tile

    f8 = mybir.dt.float8e4
    bf = mybir.dt.bfloat16
    f16 = mybir.dt.float16
    f32 = mybir.dt.float32
    DR = mybir.MatmulPerfMode.DoubleRow
    NL = 2 * cap1
    SL = cap1 * 8  # slot boundary in (n,d) columns

    nc = bacc.Bacc("TRN2", target_bir_lowering=False, debug=False, num_devices=NCORES)

    pis_d = nc.dram_tensor("pis", [128, 2048], f8, kind="ExternalInput")
    xt2_d = nc.dram_tensor("xt2", [128, 2 * NL * 8], f8, kind="ExternalInput")
    ytl_d = nc.dram_tensor("ytl", [128, 16 * NYL], f8, kind="ExternalInput")
    aux_d = nc.dram_tensor("aux", [4, NL + 2 * NYL], f16, kind="ExternalInput")
    out_d = nc.dram_tensor("out", [NL, NYL], bf, kind="ExternalOutput")

    NB = [(i, min(128, NL - i)) for i in range(0, NL, 128)]

    # engine split of each 2048-col ysq chunk: (ACT, DVE, rest=Pool)
    YA, YD = 900, 780
    # engine split of each xsq chunk (fractions of SL per tc)
    XA, XD = 0.40, 0.40

    with tile.TileContext(nc) as tc:
        with (
            tc.tile_pool(name="io", bufs=1) as pio,
            tc.tile_pool(name="work", bufs=1) as pw,
            tc.tile_pool(name="small", bufs=1) as psm,
            tc.tile_pool(name="ps", bufs=1, space="PSUM") as pp,
        ):
            pis = pio.tile([128, 2048], f8, tag="pis")
            xt2 = pio.tile([128, 2 * NL * 8], f8, tag="xt2")
            ytl = pio.tile([128, 16 * NYL], f8, tag="ytl")
            aux = psm.tile([4, NL + 2 * NYL], f16, tag="aux")
            xt2v = xt2.rearrange("l (t nd) -> l t nd", t=2)
            xt2dv = xt2_d.rearrange("l (t nd) -> l t nd", t=2)
            ytlv = ytl.rearrange("l (kc j) -> l kc j", kc=16)

            # ---- input DMAs: pis/xt2/aux on SP HWDGE, ytl on Pool SWDGE ---
            def ydma(q, eng):
                eng.dma_start(
                    ytl[:, q * 4 * NYL : (q + 1) * 4 * NYL],
                    ytl_d[:, q * 4 * NYL : (q + 1) * 4 * NYL],
                )

            nc.sync.dma_start(pis[:], pis_d[:, :])
            ydma(0, nc.gpsimd)
            ydma(2, nc.gpsimd)
            nc.sync.dma_start(xt2v[:, :, 0:SL], xt2dv[:, :, 0:SL])
            ydma(1, nc.sync)
            nc.sync.dma_start(xt2v[:, :, SL:], xt2dv[:, :, SL:])
            ydma(3, nc.sync)
            nc.sync.dma_start(aux[:], aux_d[:, :])

            piSv = pis[:, 0:1024].rearrange("l (c t p) -> l c t p", c=2, t=2)
            piTv = pis[:, 1024:2048].rearrange("l (c pc t) -> l c pc t", c=2, pc=2)

            ones8 = psm.tile([128, 2], f8, tag="ones8")
            nc.vector.memset(ones8[:], 1.0)
            ones8v = ones8.rearrange("l (t o) -> l t o", o=1)
            # ACT square-table preload (1.3us) during the DMA window
            dummy8 = psm.tile([128, 2], f8, tag="dummy8")
            nc.scalar.square(dummy8[:], ones8[:])

            # ---- colsum + rowsum (PE) + evacs (DVE) -----------------------
            csrs = pp.tile([128, 16], f32, tag="pssmall", bufs=2, name="csrs")
            for c in range(2):
                for pc in range(2):
                    nc.tensor.matmul(
                        csrs[:, 2 * c + pc : 2 * c + pc + 1],
                        piSv[:, c, :, pc * 128 : (pc + 1) * 128],
                        ones8v[:],
                        start=True, stop=True, perf_mode=DR,
                        skip_group_check=True,
                    )
            for c in range(2):
                for tcc in range(2):
                    nc.tensor.matmul(
                        csrs[:, 8 + 2 * c + tcc : 8 + 2 * c + tcc + 1],
                        piTv[:, c, :, tcc * 128 : (tcc + 1) * 128],
                        ones8v[:],
                        start=True, stop=True, perf_mode=DR,
                        skip_group_check=True,
                    )
            colsR = psm.tile([128, 64], f8, tag="colsR")
            colsRv = colsR.rearrange("l (pc k c) -> l pc k c", pc=2, k=2)
            cspsv = csrs[:, 0:4].rearrange("l (c pc) -> l pc c", c=2)
            rowsS2 = psm.tile([128, 64], f8, tag="rowsS2")
            rowsS2v = rowsS2.rearrange("l (t k c) -> l t k c", t=2, k=2)
            rspsv = csrs[:, 8:12].rearrange("l (c t) -> l t c", c=2)
            for k in range(2):
                nc.vector.tensor_copy(colsRv[:, :, k, 0:2], cspsv)
                nc.vector.tensor_copy(rowsS2v[:, :, k, 0:2], rspsv)

            # ---- warp matmuls (PE), evacs deferred ------------------------
            xwt = pw.tile([128, 16 * NL], f8, tag="xwt")
            xwtv = xwt.rearrange("l (kc n) -> l kc n", kc=16)
            evacs = []
            for pc in (0, 1):
                for s in (0, 1):
                    for c0 in range(s * SL, (s + 1) * SL, 512):
                        c1 = min(c0 + 512, (s + 1) * SL)
                        w = pp.tile(
                            [128, 512], f32, tag="psxw", bufs=3,
                            name=f"xw{pc}s{s}c{c0}",
                        )
                        wv = w[:, 0 : c1 - c0]
                        nc.tensor.matmul(
                            wv,
                            piSv[:, s, :, pc * 128 : (pc + 1) * 128],
                            xt2v[:, :, c0:c1],
                            start=True, stop=True, perf_mode=DR,
                            skip_group_check=True,
                        )
                        evacs.append((wv, c0, c1, pc))

            def xw_evac(i, eng):
                wv, c0, c1, pc = evacs[i]
                dst = xwtv[:, pc * 8 : (pc + 1) * 8, c0 // 8 : c1 // 8]
                src = wv.rearrange("l (n d) -> l d n", d=8)
                if eng == "a":
                    nc.scalar.mul(dst, src, -2.0)
                elif eng == "d":
                    nc.vector.tensor_scalar_mul(dst, src, -2.0)
                else:
                    nc.gpsimd.tensor_scalar_mul(dst, src, -2.0)

            # ---- squares --------------------------------------------------
            ysq = pw.tile([128, 16 * NYL], f8, tag="ysq")
            ysqv = ysq.rearrange("l (kc j) -> l kc j", kc=16)

            def ysq_chunk(q, nq=1):
                c0 = q * 4 * NYL
                ya, yd = YA * nq, YD * nq
                nc.scalar.square(ysq[:, c0 : c0 + ya], ytl[:, c0 : c0 + ya])
                nc.vector.tensor_mul(
                    ysq[:, c0 + ya : c0 + ya + yd],
                    ytl[:, c0 + ya : c0 + ya + yd],
                    ytl[:, c0 + ya : c0 + ya + yd],
                )
                nc.gpsimd.tensor_mul(
                    ysq[:, c0 + ya + yd : c0 + nq * 4 * NYL],
                    ytl[:, c0 + ya + yd : c0 + nq * 4 * NYL],
                    ytl[:, c0 + ya + yd : c0 + nq * 4 * NYL],
                )

            xsq = pw.tile([128, 2 * NL * 8], f8, tag="xsq")
            xsqv = xsq.rearrange("l (t nd) -> l t nd", t=2)

            def xsq_chunk(s):
                h0, h1 = s * SL, (s + 1) * SL
                aa = int(SL * XA)
                dd = int(SL * XD)
                nc.scalar.square(
                    xsqv[:, :, h0 : h0 + aa], xt2v[:, :, h0 : h0 + aa]
                )
                nc.vector.tensor_mul(
                    xsqv[:, :, h0 + aa : h0 + aa + dd],
                    xt2v[:, :, h0 + aa : h0 + aa + dd],
                    xt2v[:, :, h0 + aa : h0 + aa + dd],
                )
                nc.gpsimd.tensor_mul(
                    xsqv[:, :, h0 + aa + dd : h1],
                    xt2v[:, :, h0 + aa + dd : h1],
                    xt2v[:, :, h0 + aa + dd : h1],
                )

            E = ("a", "d", "p") if POOL_PSUM else ("a", "d", "a", "d")
            ysq_chunk(0)
            xsq_chunk(0)
            xw_evac(0, E[0])
            xw_evac(1, E[1])
            xw_evac(2, E[2])
            ysq_chunk(1)
            xw_evac(3, E[3])
            xw_evac(4, E[0])
            xw_evac(5, E[1])
            xsq_chunk(1)
            xw_evac(6, E[2])
            xw_evac(7, E[3])
            xw_evac(8, E[0])
            ysq_chunk(2)
            xw_evac(9, E[1])
            xw_evac(10, E[2])
            xw_evac(11, E[3])
            ysq_chunk(3)

            # ---- C2 chain + C3 (PE) ---------------------------------------
            c2ps = pp.tile([2, NYL], f32, tag="pssmall", bufs=2, name="c2ps")

            def c2dr(pc, m, start, stop):
                kc = pc * 8 + 2 * m
                nc.tensor.matmul(
                    c2ps[:],
                    colsRv[:, pc, :, 0:2],
                    ysqv[:, kc : kc + 2, :],
                    start=start, stop=stop, perf_mode=DR,
                    skip_group_check=True,
                )

            outps = [
                pp.tile([pn, NYL], f32, tag="psout", bufs=3, name=f"outps{ib}")
                for ib, (i0, pn) in enumerate(NB)
            ]

            def c3dr(pc, m, start):
                kc = pc * 8 + 2 * m
                for ib, (i0, pn) in enumerate(NB):
                    nc.tensor.matmul(
                        outps[ib][:],
                        xwtv[:, kc : kc + 2, i0 : i0 + pn],
                        ytlv[:, kc : kc + 2, :],
                        start=start, stop=False, perf_mode=DR,
                        skip_group_check=True,
                    )

            for m in range(4):
                c2dr(0, m, m == 0, False)
            for m in range(4):
                c3dr(0, m, m == 0)
            c3dr(1, 0, False)
            c3dr(1, 1, False)
            c2dr(1, 0, False, False)
            c2dr(1, 1, False, False)

            # C1 chain (needs xsq complete + rowsS2)
            xsq4 = xsq.rearrange("l (t n d) -> l t d n", t=2, d=8)
            c1ps = pp.tile([2, NL], f32, tag="pssmall", bufs=2, name="c1ps")
            ci = 0
            for tcc in range(2):
                for m in range(4):
                    nc.tensor.matmul(
                        c1ps[:],
                        rowsS2v[:, tcc, :, 0:2],
                        xsq4[:, tcc, 2 * m : 2 * m + 2, :],
                        start=(ci == 0), stop=(ci == 7), perf_mode=DR,
                        skip_group_check=True,
                    )
                    ci += 1

            # c1c evac, augB = ind*c1c, DMA into aux rows 2-3 (off-tail)
            c1c = psm.tile([2, NL], f16, tag="c1c")
            nc.scalar.mul(c1c[:], c1ps[:], 1.0)
            augB = psm.tile([2, NL], f16, tag="augB")
            nc.vector.tensor_mul(augB[:], aux[0:2, 0:NL], c1c[:])

            c2dr(1, 2, False, False)
            c2dr(1, 3, False, True)
            c3dr(1, 2, False)
            c3dr(1, 3, False)

            # ---- tail: c2row evac into aux, rank-4 augs, out --------------
            nc.vector.tensor_copy(aux[0:2, NL : NL + NYL], c2ps[:])

            outsb = pw.tile([128, 3 * NYL], bf, tag="outsb")
            for ib, (i0, pn) in enumerate(NB):
                nc.tensor.matmul(
                    outps[ib][:],
                    augB[:, i0 : i0 + pn],
                    aux[0:2, NL + NYL : NL + 2 * NYL],
                    start=False, stop=False,
                    skip_group_check=True,
                )
            for ib, (i0, pn) in enumerate(NB):
                nc.tensor.matmul(
                    outps[ib][:],
                    aux[0:2, i0 : i0 + pn],
                    aux[0:2, NL : NL + NYL],
                    start=False, stop=True,
                    skip_group_check=True,
                )
                dst = outsb[0:pn, ib * NYL : (ib + 1) * NYL]
                if ib == 0:
                    nc.vector.tensor_copy(dst, outps[ib][:])
                elif ib == 1:
                    nc.scalar.mul(dst, outps[ib][:], 1.0)
                else:
                    nc.vector.tensor_copy(dst, outps[ib][:])
                (nc.scalar if ib == 0 else nc.sync).dma_start(
                    out_d[i0 : i0 + pn, :], dst
                )

    nc.compile()
    return nc


def kernel(X, Y, pi_dtw, classes):
    import ml_dtypes
    from concourse.bass_utils import run_bass_kernel_spmd

    f8 = ml_dtypes.float8_e4m3
    X = np.ascontiguousarray(np.asarray(X, dtype=np.float32))
    Y = np.ascontiguousarray(np.asarray(Y, dtype=np.float32))
    pi_dtw = np.ascontiguousarray(np.asarray(pi_dtw, dtype=np.float32))
    classes = np.asarray(classes).astype(np.int64)

    counts = np.bincount(classes, minlength=C)
    cap1 = int(-(-int(counts.max()) // 16) * 16)
    NL = 2 * cap1

    if cap1 not in _cache:
        _cache[cap1] = _build(cap1)
    nc = _cache[cap1]

    idx = [np.nonzero(classes == c)[0] for c in range(C)]

    ytls = []
    for cj in range(2):
        Yh = Y[cj * NYL : (cj + 1) * NYL]
        B = Yh.transpose(1, 2, 0).reshape(2, 128, 8, NYL)
        ytls.append(
            np.ascontiguousarray(
                B.transpose(1, 0, 2, 3).reshape(128, 16 * NYL)
            ).astype(f8)
        )

    in_maps = []
    for r in range(4):
        ca, cb = 2 * r, 2 * r + 1
        Xp = np.zeros((NL, T, D), dtype=np.float32)
        Xp[0 : counts[ca]] = X[idx[ca]]
        Xp[cap1 : cap1 + counts[cb]] = X[idx[cb]]
        A = Xp.transpose(1, 0, 2).reshape(2, 128, NL, D)
        xt2 = np.ascontiguousarray(
            A.transpose(1, 0, 2, 3).reshape(128, 2 * NL * D)
        ).astype(f8)

        P = pi_dtw[[ca, cb]]
        piS = P.reshape(2, 2, 128, 256).transpose(2, 0, 1, 3).reshape(128, 1024)
        PT = np.ascontiguousarray(P.transpose(0, 2, 1))
        piT = PT.reshape(2, 2, 128, 256).transpose(2, 0, 1, 3).reshape(128, 1024)
        pis = np.ascontiguousarray(
            np.concatenate([piS, piT], axis=1)
        ).astype(f8)

        aux = np.zeros((4, NL + 2 * NYL), dtype=np.float16)
        aux[0, 0:cap1] = 1.0
        aux[1, cap1:NL] = 1.0
        aux[0:2, NL + NYL :] = 1.0

        for cj in range(2):
            in_maps.append(
                {"pis": pis, "xt2": xt2, "ytl": ytls[cj], "aux": aux}
            )

    res = run_bass_kernel_spmd(nc, in_maps, core_ids=list(range(NCORES)))

    out = np.empty((N, NY), dtype=np.float32)
    jr = [np.arange(0, NYL), np.arange(NYL, NY)]
    for r in range(4):
        ca, cb = 2 * r, 2 * r + 1
        for cj in range(2):
            blk = np.asarray(res.results[2 * r + cj]["out"]).astype(np.float32)
            out[np.ix_(idx[ca], jr[cj])] = blk[0 : counts[ca]]
            out[np.ix_(idx[cb], jr[cj])] = blk[cap1 : cap1 + counts[cb]]
    return out



# revision 3
# speedup vs baseline: 1.1898x; 1.1898x over previous
"""Trainium2 Bass kernel for the CNN-MAD per-class DTW transport cost.

Math (reference):
  mat_cost[n, j] = C1[n] + C2[c_n, j] - 2*C3[n, j],  c_n = classes[n]
    C1[n]    = sum_t rowsum[c_n, t] * r[n,t],   r[n,t] = sum_d X[n,t,d]^2
    C2[c, j] = sum_p colsum[c, p]  * q[j,p],    q[j,p] = sum_d Y[j,p,d]^2
    C3[n, j] = sum_{p,d} XW[n,p,d] * Y[j,p,d],  XW = pi_c.T @ X (warp)

Sharding: 4x2 grid. Core (rr, cj) owns the samples of classes {2rr, 2rr+1}
(zero-padded to cap1 rows per class, NL = 2*cap1) and the j-half
[512*cj, 512*(cj+1)).  One SPMD program for all 8 cores; per-core class
structure enters only through data.  Host ships fp8 row-norms q/r and the
tiny pi col/row sums; the device runs all four contractions on the PE:
  - warp XW = piS.T @ X at fp8 DoubleRow rate, psum evac'd as a pure
    contiguous copy ((d,n)-major layout, -2 prefolded into the shipped Y).
  - C3 flipped to [j-partition, n-free] psum orientation: 4 j-blocks of
    128, 8 DR passes each over k=(p,d); cost scales with n=NL not NY.
  - C1/C2 as single DR matmuls against r/q; their rows enter each output
    psum via one rank-3 fp16 augmentation matmul per j-block (issued
    early, as the psum group starter).
  - outputs leave via SWDGE prepare/trigger writebacks (per j-block
    queues) so the post-compute DMA latency is trigger+transfer+sem only.
"""

import sys

sys.path.insert(0, "/opt/trn_rl_repo")

import numpy as np

N, NY, T, TP, D, C = 1024, 1024, 256, 256, 8, 8
NCORES = 8
NYL = 512  # j columns per core

_cache = {}

# engines for the 12 warp-psum evacs (rotation), the 4 out evacs, c1/c2
XW_EVAC = ("a", "d", "p")
OUT_EVAC = ("a", "d", "a", "d")
N_PRIME = 0  # PE p-state priming matmuls (0 = off)
WB_JBS = ()  # j-blocks whose output goes via prepare/trigger writeback


def _copy(nc, eng, dst, src):
    if eng == "a":
        nc.scalar.mul(dst, src, 1.0)
    elif eng == "d":
        nc.vector.tensor_copy(dst, src)
    else:
        nc.gpsimd.tensor_copy(dst, src)


def _build(cap1):
    import concourse.bacc as bacc
    import concourse.mybir as mybir
    import concourse.tile as tile

    f8 = mybir.dt.float8e4
    bf = mybir.dt.bfloat16
    f16 = mybir.dt.float16
    f32 = mybir.dt.float32
    i32 = mybir.dt.int32
    DR = mybir.MatmulPerfMode.DoubleRow
    NL = 2 * cap1

    # pqs column map (fp8): piS | q | r | colsum | rowsum
    QO = 1024          # q offset
    RO = QO + 1024     # r offset
    CO = RO + 2 * NL   # colsum offset [pc, 3]
    WO = CO + 6        # rowsum offset [tc, 2]
    PQS = WO + 4

    nwb = len(WB_JBS)
    nc = bacc.Bacc(
        "TRN2",
        target_bir_lowering=False,
        debug=False,
        num_devices=NCORES,
        num_swdge_queues=max(1, nwb),
    )

    pqs_d = nc.dram_tensor("pqs", [128, PQS], f8, kind="ExternalInput")
    xt2_d = nc.dram_tensor("xt2", [128, 16 * NL], f8, kind="ExternalInput")
    ytl_d = nc.dram_tensor("ytl", [128, 16 * NYL], f8, kind="ExternalInput")
    aux_d = nc.dram_tensor("aux", [4, NYL + NL + 16], f16, kind="ExternalInput")
    out_d = nc.dram_tensor("out", [NYL, NL], bf, kind="ExternalOutput")

    with tile.TileContext(nc) as tc:
        with (
            tc.tile_pool(name="io", bufs=1) as pio,
            tc.tile_pool(name="work", bufs=1) as pw,
            tc.tile_pool(name="small", bufs=1) as psm,
            tc.tile_pool(name="ps", bufs=1, space="PSUM") as pp,
        ):
            pqs = pio.tile([128, PQS], f8, tag="pqs")
            xt2 = pio.tile([128, 16 * NL], f8, tag="xt2")
            ytl = pio.tile([128, 16 * NYL], f8, tag="ytl")
            aux = psm.tile([4, NYL + NL + 16], f16, tag="aux")

            piSv = pqs[:, 0:1024].rearrange("l (c t p) -> l c t p", c=2, t=2)
            qv = pqs[:, QO:RO].rearrange("l (pc j) -> l pc j", pc=2)
            rv = pqs[:, RO:CO].rearrange("l (tc n) -> l tc n", tc=2)
            csv = pqs[:, CO:WO].rearrange("l (pc c) -> l pc c", pc=2)
            rsv = pqs[:, WO : WO + 4].rearrange("l (tc c) -> l tc c", tc=2)
            xt2v = xt2.rearrange("l (t d n) -> l t d n", t=2, d=8)
            ytlv = ytl.rearrange("l (jb kc j) -> l jb kc j", jb=4, kc=16)

            augL = aux[0:3, 0:NYL]            # [ones | c2A | c2B] over j
            augR = aux[0:3, NYL : NYL + NL]   # [c1c | indA | indB] over n

            # ---- writeback preps (descriptor gen; data read at trigger) ---
            wb_sems = {}
            if nwb:
                idxs = psm.tile([128, 2], i32, tag="wbidx")
                nc.gpsimd.memset(idxs[:], 0)
                outsb = pw.tile([128, 4 * NL], bf, tag="outsb")
                outv = outsb.rearrange("j (jb o b n) -> j jb o b n", jb=4, o=1, b=2)
                odv = out_d.rearrange("(jb j o) (b n) -> jb b j o n", jb=4, o=1, b=2)
                for jb in sorted(WB_JBS):
                    qn = sorted(WB_JBS).index(jb)
                    sem = nc.alloc_semaphore(f"wbdma{jb}")
                    wb_sems[jb] = sem
                    nc.gpsimd.kv_writeback(
                        odv[jb],
                        outv[:, jb],
                        idxs[:],
                        prepare_only=True,
                        sem=sem,
                        queue_num=qn,
                    )
            else:
                outsb = pw.tile([128, 4 * NL], bf, tag="outsb")

            # ---- input DMAs (SP HWDGE; aux via ACT queue) -----------------
            nc.sync.dma_start(pqs[:], pqs_d[:, :])
            nc.scalar.dma_start(aux[:], aux_d[:, :])
            nc.sync.dma_start(xt2v[:, :, 0:4, :], xt2_d.rearrange("l (t d n) -> l t d n", t=2, d=8)[:, :, 0:4, :])
            nc.sync.dma_start(xt2v[:, :, 4:8, :], xt2_d.rearrange("l (t d n) -> l t d n", t=2, d=8)[:, :, 4:8, :])
            ytldv = ytl_d.rearrange("l (jb x) -> l jb x", jb=4)
            ytlsv = ytl.rearrange("l (jb x) -> l jb x", jb=4)
            for jb in range(4):
                nc.sync.dma_start(ytlsv[:, jb], ytldv[:, jb])

            # ---- PE p-state priming (dummy matmuls on scratch) ------------
            if N_PRIME:
                dum = psm.tile([128, 2], f8, tag="dum")
                nc.vector.memset(dum[:], 1.0)
                dumv = dum.rearrange("l (t o) -> l t o", o=1)
                for i in range(N_PRIME):
                    w = pp.tile([128, 512], f32, tag="psW", bufs=3, name=f"du{i}")
                    nc.tensor.matmul(
                        w[:, 0:1], dumv, dumv,
                        start=True, stop=True, perf_mode=DR,
                        skip_group_check=True,
                    )

            # ---- C2 / C1 (DR) + evacs into aug rows -----------------------
            ccps = pp.tile([3, NYL], f32, tag="psS", bufs=1, name="ccps")
            nc.tensor.matmul(
                ccps[:], csv, qv,
                start=True, stop=True, perf_mode=DR, skip_group_check=True,
            )
            for s in range(2):
                nc.tensor.matmul(
                    ccps[0:1, s * cap1 : (s + 1) * cap1],
                    rsv[:, :, s : s + 1],
                    rv[:, :, s * cap1 : (s + 1) * cap1],
                    start=True, stop=True, perf_mode=DR, skip_group_check=True,
                )
            nc.vector.tensor_copy(aux[1:3, 0:NYL], ccps[1:3, :])
            nc.scalar.mul(aux[0:1, NYL : NYL + NL], ccps[0:1, 0:NL], 1.0)

            # ---- aug matmuls: psum group starters -------------------------
            outps = [
                pp.tile([128, NL], f32, tag="psO", bufs=4, name=f"outps{jb}")
                for jb in range(4)
            ]
            for jb in range(4):
                nc.tensor.matmul(
                    outps[jb][:],
                    augL[:, jb * 128 : (jb + 1) * 128],
                    augR,
                    start=True, stop=False, skip_group_check=True,
                )

            # ---- warp (PE) + contiguous evacs -----------------------------
            xwt = pw.tile([128, 16 * NL], f8, tag="xwt")
            units = [(pc, d, s) for pc in range(2) for d in range(8) for s in range(2)]
            ei = 0
            for t0 in range(0, len(units), 3):
                grp = units[t0 : t0 + 3]
                w = pp.tile(
                    [128, len(grp) * cap1], f32, tag="psW", bufs=3, name=f"xw{t0}"
                )
                for u, (pc, d, s) in enumerate(grp):
                    nc.tensor.matmul(
                        w[:, u * cap1 : (u + 1) * cap1],
                        piSv[:, s, :, pc * 128 : (pc + 1) * 128],
                        xt2v[:, :, d, s * cap1 : (s + 1) * cap1],
                        start=True, stop=True, perf_mode=DR,
                        skip_group_check=True,
                    )
                c0 = t0 * cap1
                _copy(nc, XW_EVAC[ei % len(XW_EVAC)],
                      xwt[:, c0 : c0 + len(grp) * cap1], w[:])
                ei += 1
            xwtv = xwt.rearrange("l (kc n) -> l kc n", kc=16)

            # ---- C3: 8 DR passes per j-block ------------------------------
            for jb in range(4):
                for k in range(8):
                    nc.tensor.matmul(
                        outps[jb][:],
                        ytlv[:, jb, 2 * k : 2 * k + 2, :],
                        xwtv[:, 2 * k : 2 * k + 2, :],
                        start=False, stop=(k == 7), perf_mode=DR,
                        skip_group_check=True,
                    )
                _copy(nc, OUT_EVAC[jb],
                      outsb[:, jb * NL : (jb + 1) * NL], outps[jb][:])
                if jb in wb_sems:
                    qn = sorted(WB_JBS).index(jb)
                    nc.gpsimd.trigger_dma(count=None, queue_num=qn)
                else:
                    nc.sync.dma_start(
                        out_d[jb * 128 : (jb + 1) * 128, :],
                        outsb[:, jb * NL : (jb + 1) * NL],
                    )

    nc.compile()
    return nc


def kernel(X, Y, pi_dtw, classes):
    import ml_dtypes
    from concourse.bass_utils import run_bass_kernel_spmd

    f8 = ml_dtypes.float8_e4m3
    X = np.ascontiguousarray(np.asarray(X, dtype=np.float32))
    Y = np.ascontiguousarray(np.asarray(Y, dtype=np.float32))
    pi_dtw = np.ascontiguousarray(np.asarray(pi_dtw, dtype=np.float32))
    classes = np.asarray(classes).astype(np.int64)

    counts = np.bincount(classes, minlength=C)
    cap1 = int(-(-int(counts.max()) // 16) * 16)
    NL = 2 * cap1

    if cap1 not in _cache:
        _cache[cap1] = _build(cap1)
    nc = _cache[cap1]

    idx = [np.nonzero(classes == c)[0] for c in range(C)]

    # per j-half: ytl (-2Y, [p_in, jb, pc, d, jj]) and q ([p_in, pc, j])
    ytls, qs = [], []
    qfull = (Y * Y).sum(axis=2)  # [NY, TP]
    for cj in range(2):
        Yh = -2.0 * Y[cj * NYL : (cj + 1) * NYL]
        B = Yh.reshape(4, 128, 2, 128, D).transpose(3, 0, 2, 4, 1)
        ytls.append(np.ascontiguousarray(B.reshape(128, 16 * NYL)).astype(f8))
        qh = qfull[cj * NYL : (cj + 1) * NYL]  # [512, 256]
        qs.append(
            np.ascontiguousarray(
                qh.T.reshape(2, 128, NYL).transpose(1, 0, 2).reshape(128, 2 * NYL)
            ).astype(f8)
        )

    rfull = (X * X).sum(axis=2)  # [N, T]
    colsum = pi_dtw.sum(axis=1)  # [C, TP]
    rowsum = pi_dtw.sum(axis=2)  # [C, T]

    in_maps = []
    for r in range(4):
        ca, cb = 2 * r, 2 * r + 1
        Xp = np.zeros((NL, T, D), dtype=np.float32)
        Xp[0 : counts[ca]] = X[idx[ca]]
        Xp[cap1 : cap1 + counts[cb]] = X[idx[cb]]
        xt2 = np.ascontiguousarray(
            Xp.reshape(NL, 2, 128, D).transpose(2, 1, 3, 0).reshape(128, 16 * NL)
        ).astype(f8)

        P = pi_dtw[[ca, cb]]
        piS = P.reshape(2, 2, 128, 256).transpose(2, 0, 1, 3).reshape(128, 1024)

        rp = np.zeros((NL, T), dtype=np.float32)
        rp[0 : counts[ca]] = rfull[idx[ca]]
        rp[cap1 : cap1 + counts[cb]] = rfull[idx[cb]]
        rl = rp.T.reshape(2, 128, NL).transpose(1, 0, 2).reshape(128, 2 * NL)

        cs = np.zeros((128, 2, 3), dtype=np.float32)
        cs[:, :, 1] = colsum[ca].reshape(2, 128).T
        cs[:, :, 2] = colsum[cb].reshape(2, 128).T
        rs = np.zeros((128, 2, 2), dtype=np.float32)
        rs[:, :, 0] = rowsum[ca].reshape(2, 128).T
        rs[:, :, 1] = rowsum[cb].reshape(2, 128).T

        aux = np.zeros((4, NYL + NL + 16), dtype=np.float16)
        aux[0, 0:NYL] = 1.0  # ones row of augL
        aux[1, NYL : NYL + counts[ca]] = 1.0  # indA
        aux[2, NYL + cap1 : NYL + cap1 + counts[cb]] = 1.0  # indB

        for cj in range(2):
            pqs = np.concatenate(
                [piS, qs[cj].astype(np.float32), rl,
                 cs.reshape(128, 6), rs.reshape(128, 4)],
                axis=1,
            ).astype(f8)
            in_maps.append(
                {"pqs": pqs, "xt2": xt2, "ytl": ytls[cj], "aux": aux}
            )

    res = run_bass_kernel_spmd(nc, in_maps, core_ids=list(range(NCORES)))

    out = np.empty((N, NY), dtype=np.float32)
    jr = [np.arange(0, NYL), np.arange(NYL, NY)]
    for r in range(4):
        ca, cb = 2 * r, 2 * r + 1
        for cj in range(2):
            blk = np.asarray(res.results[2 * r + cj]["out"]).astype(np.float32)
            out[np.ix_(idx[ca], jr[cj])] = blk[:, 0 : counts[ca]].T
            out[np.ix_(idx[cb], jr[cj])] = blk[:, cap1 : cap1 + counts[cb]].T
    return out
